# revision 22
# baseline (speedup 1.0000x reference)
"""Trainium2 Bass kernel for NeuralComplexityLoss (sample-entropy MSE).

Contract: kernel(predictions, targets) -> np.float32 scalar (shape ()),
matching reference.reference(). Self-contained: hardcodes shapes/sharding.

Structure (128 signals sharded 16 per core across 8 NeuronCores; per core,
per signal, upper-triangle match counts over 8 strips of 126 template rows):

  DVE  : d = x_j - x_i via tensor_scalar (per-partition f32 bias) from an
         f16 x broadcast -> f16 d (4x_2p mode; the old scalar_tensor_tensor
         path has NO fast modes, 1x).  abs/pow/mod are ISA-invalid for
         tensor_scalar, so |d| <= R is computed as d^2 <= R^2: one
         tensor_tensor mult (2x) + one tensor_scalar is_le (4x), both over
         the whole concatenated strip row.  No triangle mask anywhere.
  PE   : ps = 4*I0 + 2*I1 + 1*I2 via 3 shift-weight matmuls per 512-chunk,
         PLUS one mask-matmul per strip (moving = const identity, stationary
         = -16 * strict-upper) that adds -16 below the strip diagonal --
         replacing the old per-element mask multiply entirely.
  ACT  : q = Relu(ps - 5) in {0,1,2}, accum_out = cnt2+cnt3 per row (8 ops
         per signal, the only PSUM readers).
  DVE  : (q >= 2) with accum_out = cnt3, ONE tensor_scalar per signal (4x).

Precision: x is rounded to f16 for the streamed operand and a is stored f16
(bias stays exact f32).  Deterministic rel err vs the f32 reference is
7.9e-3 (measured on the actual seed-0 inputs), comfortably under the 2e-2
gate.  Counts themselves are exact integers in f32 PSUM/accum.

Host reduces the per-row counts, applies the 2c-N symmetry, the 14-row tail
triangle (rows >= 1008), entropies, and the final MSE.  A For_i hardware
loop (_reps) repeats the signal loop for wall-clock-differencing timing.
"""

import numpy as np

B, C, T = 4, 16, 1024
M = 2
R = 0.2
EPS = 1e-8
N = T - M                      # 1022 templates
NCORES = 8
NSIG = 2 * B * C               # 128 signals total
S_PER_CORE = NSIG // NCORES    # 16
STRIP = 126
DEV_STRIPS = 8
TAIL_R0 = STRIP * DEV_STRIPS           # 1008
PS_CHUNK = 512                 # matmul fp32 PSUM output: one bank = 512 cols

WCS = [N - STRIP * rt for rt in range(DEV_STRIPS)]        # anchors per strip
AOFF = [0]
for _wc in WCS:
    AOFF.append(AOFF[-1] + _wc + 2)                        # a/im strip offsets
ATOT = AOFF[-1]                                            # 4664
QTOT = sum(WCS)                                            # 4648

# PSUM packing groups: strips sharing one PSUM tile, extracted by ONE ACT op.
# Per-strip column offsets inside the tile must not make any matmul chunk
# cross a 512-col PSUM bank edge.
PS_GROUPS = ((0,), (1,), (2,), (3, 7), (4, 6), (5,))
GOFF = {}          # strip -> (group_idx, col offset inside group tile)
GWIDTH = []        # group -> total width
QGOFF = []         # group -> offset of group's q/extract columns
_q = 0
for _gi, _g in enumerate(PS_GROUPS):
    _off = 0
    for _rt in _g:
        GOFF[_rt] = (_gi, _off)
        _off += WCS[_rt]
    GWIDTH.append(_off)
    QGOFF.append(_q)
    _q += _off
assert _q == QTOT

_CACHE = {}
LAST_RESULTS = None


def _split_excess_waits(nc, maxw=1):
    """Walrus codegen accepts only one sync-wait per instruction: hoist
    extras onto preceding single-wait NOPs on the same engine."""
    import bass_rust
    import concourse.mybir as mybir

    n_split = 0
    for bb in nc.main_func.blocks:
        insts = bb.instructions
        i = 0
        while i < len(insts):
            ins = insts[i]
            si = ins.sync_info
            waits = list(si.on_wait) if si is not None and si.on_wait else []
            if len(waits) > maxw:
                extra, keep = waits[:-maxw], waits[-maxw:]
                nops = []
                for j, w in enumerate(extra):
                    nop = bass_rust.InstNoOp(
                        name=f"{ins.name}-wsplit{j}", ins=[], outs=[]
                    )
                    nop.engine = ins.engine
                    nop.sync_info = mybir.SyncInfo(on_wait=[w], on_update=[])
                    nops.append(nop)
                si.on_wait = keep
                insts[i:i] = nops
                i += len(nops)
                n_split += 1
            i += 1
    return n_split


def _build(reps=1, bufs=4, body_copies=1):
    import concourse.bass as bass
    import concourse.tile as tile
    from concourse import mybir
    from concourse.alu_op_type import AluOpType

    f32 = mybir.dt.float32
    f16 = mybir.dt.float16
    nc = bass.Bass(trn_type="TRN2", num_devices=NCORES)
    x = nc.dram_tensor("x", [S_PER_CORE, T], f16, kind="ExternalInput")
    xcol_d = nc.dram_tensor("xcol", [128, 128], f32, kind="ExternalInput")
    out3 = nc.dram_tensor("cnt3", [128, S_PER_CORE], f32, kind="ExternalOutput")
    out_a = nc.dram_tensor(
        "cnt_a", [128, S_PER_CORE * len(PS_GROUPS)], f32, kind="ExternalOutput"
    )

    s_np = np.zeros((3, 128, 128), dtype=np.float16)
    for k, w in enumerate((4.0, 2.0, 1.0)):   # ps = 4*I0 + 2*I1 + 1*I2
        for p in range(128 - k):
            s_np[k, p + k, p] = w
    s_dram = [nc.inline_tensor(s_np[k], name=f"shiftw{k}") for k in range(3)]
    wmask_np = (-16.0 * np.triu(np.ones((128, 128), np.float32), 1)).astype(
        np.float16
    )
    wmask_dram = nc.inline_tensor(wmask_np, name="wmask")
    ident_np = np.eye(128, dtype=np.float16)
    ident_dram = nc.inline_tensor(ident_np, name="ident")

    xa = x.ap()
    with tile.TileContext(nc) as tc:
        with (
            tc.tile_pool(name="singles", bufs=1) as singles,
            tc.tile_pool(name="xrep", bufs=bufs) as xrep,
            tc.tile_pool(name="dpool", bufs=4) as dpool,
            tc.tile_pool(name="sqpool", bufs=3) as sqpool,
            tc.tile_pool(name="impool", bufs=3) as impool,
            tc.tile_pool(name="qpool", bufs=4) as qpool,
            tc.tile_pool(name="ps", bufs=3, space="PSUM") as pspool,
            tc.tile_pool(name="psn", bufs=2, space="PSUM") as psnarrow,
        ):
            stw = singles.tile([128, 384], f16)
            for k in range(3):
                nc.sync.dma_start(
                    out=stw[:, 128 * k : 128 * k + 128], in_=s_dram[k][:, :]
                )
            wmask = singles.tile([128, 128], f16)
            nc.sync.dma_start(out=wmask, in_=wmask_dram[:, :])
            ident = singles.tile([128, 128], f16)
            nc.sync.dma_start(out=ident, in_=ident_dram[:, :])
            xcol = singles.tile([128, 128], f32)
            nc.sync.dma_start(out=xcol, in_=xcol_d[:, :])
            stats3 = singles.tile([128, S_PER_CORE], f32)
            nc.vector.memset(stats3, 0.0)
            stats_a = singles.tile([128, S_PER_CORE * len(PS_GROUPS)], f32)
            nc.vector.memset(stats_a, 0.0)
            bneg5 = singles.tile([128, 1], f32)
            nc.vector.memset(bneg5, -5.0)

            def body():
                # Signals are processed in pairs with DVE stages interleaved
                # (d ops, then the two TTs, then the two thresholds) so no
                # DVE op directly consumes the immediately preceding op's
                # output -- hides the 8-slice pipe drain between ops.
                pending = []
                for sp in range(0, S_PER_CORE, 2):
                    pair = (sp, sp + 1)
                    tl = {}
                    for s in pair:
                        row = xa[s : s + 1, :]
                        x_rep = xrep.tile([128, T], f16)
                        nc.sync.dma_start(
                            out=x_rep,
                            in_=bass.AP(
                                tensor=row.tensor,
                                offset=row.offset,
                                ap=[[0, 128], [1, T]],
                            ),
                        )
                        d_sig = dpool.tile([128, ATOT], f16)
                        sq_sig = sqpool.tile([128, ATOT], f16)
                        im_sig = impool.tile([128, ATOT], f16)
                        q_sig = qpool.tile([128, QTOT], f16)
                        tl[s] = (x_rep, d_sig, sq_sig, im_sig, q_sig)
                    for rt in range(DEV_STRIPS):
                        r0 = STRIP * rt
                        wc = WCS[rt]
                        for s in pair:
                            # d = x_j - x_i (per-partition f32 bias, 4x)
                            nc.vector.tensor_scalar(
                                out=tl[s][1][:, AOFF[rt] : AOFF[rt] + wc + 2],
                                in0=tl[s][0][:, r0 : r0 + wc + 2],
                                scalar1=xcol[:, 8 * s + rt : 8 * s + rt + 1],
                                scalar2=None,
                                op0=AluOpType.subtract,
                            )
                    for s in pair:
                        # d^2 (2x mode)
                        nc.vector.tensor_tensor(
                            out=tl[s][2][:, 0:ATOT],
                            in0=tl[s][1][:, 0:ATOT],
                            in1=tl[s][1][:, 0:ATOT],
                            op=AluOpType.mult,
                        )
                    for s in pair:
                        # im = (d^2 <= R^2) (4x mode)
                        nc.vector.tensor_scalar(
                            out=tl[s][3][:, 0:ATOT],
                            in0=tl[s][2][:, 0:ATOT],
                            scalar1=R * R,
                            scalar2=None,
                            op0=AluOpType.is_le,
                        )
                    NG = len(PS_GROUPS)
                    for s in pair:
                        im_sig, q_sig = tl[s][3], tl[s][4]
                        for gi, grp in enumerate(PS_GROUPS):
                            gw = GWIDTH[gi]
                            if gw <= PS_CHUNK:
                                ps = psnarrow.tile([128, 512], mybir.dt.float32)
                            else:
                                ps = pspool.tile([128, 1024], mybir.dt.float32)
                            for rt in grp:
                                wc = WCS[rt]
                                off = AOFF[rt]
                                poff = GOFF[rt][1]
                                for c0 in range(0, wc, PS_CHUNK):
                                    cw = min(PS_CHUNK, wc - c0)
                                    for k in (0, 1, 2):
                                        nc.tensor.matmul(
                                            ps[
                                                0:STRIP,
                                                poff + c0 : poff + c0 + cw,
                                            ],
                                            stw[0:128, 128 * k : 128 * k + STRIP],
                                            im_sig[
                                                0:128,
                                                off + c0 + k : off + c0 + k + cw,
                                            ],
                                            start=(k == 0),
                                            stop=(k == 2 and c0 > 0),
                                            skip_group_check=len(grp) > 1,
                                        )
                                    if c0 == 0:
                                        # -16 below the strip diagonal
                                        # (replaces any per-element mask)
                                        nc.tensor.matmul(
                                            ps[0:STRIP, poff : poff + 128],
                                            wmask[0:128, 0:STRIP],
                                            ident[0:128, 0:128],
                                            start=False,
                                            stop=True,
                                            skip_group_check=True,
                                        )
                            # q = Relu(ps - 5) in {0,1,2}; accum = cnt2+cnt3
                            # summed over the whole group (decode sums strips
                            # anyway)
                            nc.scalar.activation(
                                out=q_sig[0:STRIP, QGOFF[gi] : QGOFF[gi] + gw],
                                in_=ps[0:STRIP, 0:gw],
                                func=mybir.ActivationFunctionType.Relu,
                                bias=bneg5[0:STRIP, 0:1],
                                scale=1.0,
                                accum_out=stats_a[
                                    0:STRIP, NG * s + gi : NG * s + gi + 1
                                ],
                            )

                        # cnt3 = #(q >= 2), one whole-signal op (4x mode).
                        # Deferred a full pair so the DVE never waits on the
                        # ACT extracts; scratch out reuses d_sig.
                        def emit_cnt3(s=s, q_sig=q_sig, d_sig=tl[s][1]):
                            nc.vector.tensor_scalar(
                                out=d_sig[0:STRIP, 0:QTOT],
                                in0=q_sig[0:STRIP, 0:QTOT],
                                scalar1=2.0,
                                scalar2=0.0,
                                op0=AluOpType.is_ge,
                                op1=AluOpType.add,
                                accum_out=stats3[0:STRIP, s : s + 1],
                            )

                        pending.append(emit_cnt3)
                        if len(pending) > 2:
                            pending.pop(0)()
                while pending:
                    pending.pop(0)()

            if reps > 1:
                with tc.For_i(0, reps):
                    for _ in range(body_copies):
                        body()
            else:
                body()

            nc.sync.dma_start(out=out3[:, :], in_=stats3)
            nc.sync.dma_start(out=out_a[:, :], in_=stats_a)

    _split_excess_waits(nc)
    return nc


def _get_nc(reps=1):
    key = ("nc", reps)
    if key not in _CACHE:
        _CACHE[key] = _build(reps)
    return _CACHE[key]


def _get_runner(reps=1):
    """Cached jitted 8-core executor:
    (x16 [128, T] f16, xcol [8*128, 128] f32) -> list of gathered outputs."""
    key = ("fn", reps)
    if key in _CACHE:
        return _CACHE[key]
    import jax
    import numpy as _np
    from jax.sharding import Mesh, PartitionSpec
    from jax.experimental.shard_map import shard_map
    import concourse.mybir as mybir
    from concourse.bass2jax import (
        _bass_exec_p,
        install_neuronx_cc_hook,
        partition_id_tensor,
    )

    nc = _get_nc(reps)
    install_neuronx_cc_hook()

    in_names, out_names, out_avals, zero_outs = [], [], [], []
    partition_name = nc.partition_id_tensor.name if nc.partition_id_tensor else None
    for alloc in nc.m.functions[0].allocations:
        if not isinstance(alloc, mybir.MemoryLocationSet):
            continue
        name = alloc.memorylocations[0].name
        if alloc.kind == "ExternalInput":
            if name != partition_name:
                in_names.append(name)
        elif alloc.kind == "ExternalOutput":
            shape = tuple(alloc.tensor_shape)
            dtype = mybir.dt.np(alloc.dtype)
            out_names.append(name)
            out_avals.append(jax.core.ShapedArray(shape, dtype))
            zero_outs.append(_np.zeros(shape, dtype))
    n_params = len(in_names)
    n_outs = len(out_avals)
    all_in_names = list(in_names) + list(out_names) + (
        [partition_name] if partition_name else []
    )

    def _body(*args):
        operands = list(args)
        if partition_name is not None:
            operands.append(partition_id_tensor())
        return tuple(
            _bass_exec_p.bind(
                *operands,
                out_avals=tuple(out_avals),
                in_names=tuple(all_in_names),
                out_names=tuple(out_names),
                lowering_input_output_aliases=(),
                sim_require_finite=True,
                sim_require_nnan=True,
                nc=nc,
            )
        )

    devices = jax.devices("axon")[:NCORES]
    mesh = Mesh(np.asarray(devices), ("core",))
    in_specs = (PartitionSpec("core"),) * (n_params + n_outs)
    out_specs = (PartitionSpec("core"),) * n_outs
    fn = jax.jit(
        shard_map(
            _body, mesh=mesh, in_specs=in_specs, out_specs=out_specs, check_rep=False
        ),
        keep_unused=True,
    )
    concat_zeros = [
        np.zeros((NCORES * z.shape[0], *z.shape[1:]), z.dtype) for z in zero_outs
    ]
    name_order = list(in_names)

    def run(x16, xcol):
        ins = {"x": x16, "xcol": xcol}
        args = [ins[n] for n in name_order]
        res = fn(*args, *concat_zeros)
        by_name = {n: np.asarray(r) for n, r in zip(out_names, res)}
        return by_name

    _CACHE[key] = run
    return run


def _host_tail_counts(xhat):
    """Triangle counts for the tail rows [TAIL_R0, N), done on host."""
    r0 = TAIL_R0                        # 1008
    nrows = N - r0                      # 14
    blk = xhat[:, r0:T]                 # [NSIG, 16]
    D = np.abs(blk[:, :, None] - blk[:, None, :])
    I = D <= np.float32(R)
    C2 = I[:, :nrows, :nrows] & I[:, 1 : nrows + 1, 1 : nrows + 1]
    C3 = C2 & I[:, 2 : nrows + 2, 2 : nrows + 2]
    triu = np.triu(np.ones((nrows, nrows), dtype=bool))
    return (C2 & triu).sum((1, 2)).astype(np.float64), (C3 & triu).sum(
        (1, 2)
    ).astype(np.float64)


def _decode(by_name, xhat):
    """device stats -> entropies [NSIG] (float64)."""
    t2, t3 = _host_tail_counts(xhat)
    s3 = by_name["cnt3"].reshape(NCORES, 128, S_PER_CORE).astype(np.float64)
    sa = (
        by_name["cnt_a"]
        .reshape(NCORES, 128, S_PER_CORE * len(PS_GROUPS))
        .astype(np.float64)
    )
    ents = np.zeros(NSIG, dtype=np.float64)
    for c in range(NCORES):
        for s in range(S_PER_CORE):
            g = c * S_PER_CORE + s
            cnt3 = t3[g] + s3[c, 0:STRIP, s].sum()
            NG = len(PS_GROUPS)
            s1 = sa[c, 0:STRIP, NG * s : NG * (s + 1)].sum()
            cnt2 = t2[g] + (s1 - (cnt3 - t3[g]))
            m = 2.0 * cnt2 - N
            m1 = 2.0 * cnt3 - N
            ratio = m1 / max(m, 1.0)
            ent = -np.log(max(ratio, 1e-30)) if (m > 0 and m1 > 0) else 0.0
            ents[g] = ent
    return ents


def kernel(predictions, targets, _trace=False, _reps=1):
    global LAST_RESULTS

    preds = np.asarray(predictions, dtype=np.float32).reshape(B * C, T)
    targs = np.asarray(targets, dtype=np.float32).reshape(B * C, T)
    xall = np.concatenate([preds, targs], axis=0)  # [128, T]

    mu = xall.mean(axis=1, dtype=np.float64)
    sd = xall.std(axis=1, ddof=1, dtype=np.float64)
    xhat = ((xall - mu[:, None]) / (sd[:, None] + EPS)).astype(np.float32)

    x16 = np.ascontiguousarray(xhat.astype(np.float16))
    # xcol[core][p, 8*s + rt] = xhat[core*16 + s, 126*rt + p]  (exact f32 bias)
    xcol = np.zeros((NCORES, 128, 128), dtype=np.float32)
    for c in range(NCORES):
        for s in range(S_PER_CORE):
            for rt in range(DEV_STRIPS):
                r0 = STRIP * rt
                xcol[c, :, DEV_STRIPS * s + rt] = xhat[
                    c * S_PER_CORE + s, r0 : r0 + 128
                ]
    xcol = np.ascontiguousarray(xcol.reshape(NCORES * 128, 128))

    run = _get_runner(_reps)
    by_name = run(x16, xcol)
    LAST_RESULTS = by_name

    ents = _decode(by_name, xhat)
    ep = ents[: B * C].reshape(B, C)
    et = ents[B * C :].reshape(B, C)
    return np.array(np.mean((ep - et) ** 2), dtype=np.float32)
